# revision 1
# baseline (speedup 1.0000x reference)
"""MergedEmbeddingBag forward (sum pooling) on 8 Trainium2 NeuronCores.

Strategy (table-parallel, per sharding hint): core t owns table t.
Per core: for each window of 128 bags, one SWDGE indirect DMA gathers the
window's embedding rows from HBM directly into a bag-major SBUF layout
([bag_partition, item_slot, 128 floats]); a single strided DVE reduce sums
the item slots; the pooled [128, 128] tile is DMA'd back out.  No matmuls,
no on-chip index math - the per-bag index lists are precomputed on the host
(cheap: one reshape for the fixed-bag-size case) and streamed in as data, so
one static program serves all 8 cores SPMD.

Variable-size bags (general `offsets`) are handled by padding every bag in a
window to the window's max length with an index that points at an appended
all-zeros row of the weight table.
"""

import sys

sys.path.insert(0, "/opt/trn_rl_repo")

import numpy as np

# Problem geometry (hardcoded per contract; the builder itself is generic).
T = 8
N = 100000
D = 128
B = 16384
TOTAL = 327680
P = 128  # partitions / bags per window
W = B // P  # 128 windows


def _build_program(n_rows, d, n_win, lws, col_ofs, sum_l, g_bufs=6, o_bufs=4):
    """Build the SPMD raw-Bass program (explicit semaphores).

    Pipeline: gpsimd issues SWDGE indirect gathers (bag-major into SBUF),
    DVE does one strided reduce per window, SP (sync) stores pooled tiles.

    n_rows: rows in the (possibly zero-row-extended) weight table
    lws[w]: items per bag in window w (uniform within a window, padded)
    col_ofs[w]: column offset of window w's index block in the idx buffer
    sum_l: total index columns (sum of lws)
    """
    import concourse.bass as bass
    import concourse.mybir as mybir

    lmax = max(lws)
    nc = bass.Bass(num_swdge_queues=4)
    wz = nc.declare_dram_parameter("wz", [n_rows, d], mybir.dt.float32, isOutput=False)
    idx = nc.declare_dram_parameter("idx", [P, sum_l], mybir.dt.int32, isOutput=False)
    out = nc.declare_dram_parameter(
        "out", [n_win * P, d], mybir.dt.float32, isOutput=True
    )

    import contextlib

    with contextlib.ExitStack() as ctx:
        idx_sb = ctx.enter_context(nc.sbuf_tensor([P, sum_l], mybir.dt.int32))
        gbuf = ctx.enter_context(
            nc.sbuf_tensor([P, g_bufs * lmax * d], mybir.dt.float32)
        )
        obuf = ctx.enter_context(nc.sbuf_tensor([P, o_bufs * d], mybir.dt.float32))
        idx_sem = ctx.enter_context(nc.semaphore("idx_sem"))
        # One completion sem per buffer slot: at most one DMA in flight per
        # sem, so ge-16k waits are race-free.
        gsems = [ctx.enter_context(nc.semaphore(f"gsem{i}")) for i in range(g_bufs)]
        ssems = [ctx.enter_context(nc.semaphore(f"ssem{i}")) for i in range(o_bufs)]
        rsem = ctx.enter_context(nc.semaphore("rsem"))
        block = ctx.enter_context(nc.Block())

        def gslot(w):
            s = w % g_bufs
            return gbuf[:, s * lmax * d : s * lmax * d + lws[w] * d]

        def oslot(w):
            s = w % o_bufs
            return obuf[:, s * d : (s + 1) * d]

        @block.sync
        def _(sync):
            sync.dma_start(idx_sb[:], idx[:]).then_inc(idx_sem, 16)
            for w in range(n_win):
                sync.wait_ge(rsem, w + 1)
                sync.dma_start(out[w * P : (w + 1) * P, :], oslot(w)).then_inc(
                    ssems[w % o_bufs], 16
                )
            for lane in range(o_bufs):
                n_l = len(range(lane, n_win, o_bufs))
                if n_l:
                    sync.wait_ge(ssems[lane], 16 * n_l)

        # HW indirect DMA supports exactly one offset per partition per
        # instruction ([P,1] offsets -> [P,elem] dest), so a window of L
        # items takes L gather instructions.  All of window w's gathers
        # inc the window's lane sem; the consumer waits for the lane's
        # cumulative total, which is race-free because the next window on
        # a lane only starts after that wait was consumed (via rsem).
        lane_after = {}
        lane_tot = [0] * g_bufs
        for w in range(n_win):
            lane_tot[w % g_bufs] += 16 * lws[w]
            lane_after[w] = lane_tot[w % g_bufs]

        @block.gpsimd
        def _(g):
            g.wait_ge(idx_sem, 16)
            for w in range(n_win):
                if w >= g_bufs:
                    g.wait_ge(rsem, w - g_bufs + 1)
                base = (w % g_bufs) * (lmax * d)
                for l in range(lws[w]):
                    inst = g.indirect_dma_start(
                        out=gbuf[:, base + l * d : base + (l + 1) * d],
                        out_offset=None,
                        in_=wz[:],
                        in_offset=bass.IndirectOffsetOnAxis(
                            ap=idx_sb[:, col_ofs[w] + l : col_ofs[w] + l + 1],
                            axis=0,
                        ),
                    ).then_inc(gsems[w % g_bufs], 16)
                    # Spread SWDGE desc-gen across all 4 queue contexts —
                    # measured 3.6x throughput vs the single default queue.
                    q = (w * lws[w] + l) % 4
                    if q:
                        inst.ins.queue = f"qPoolDynamic{q}"

        @block.vector
        def _(v):
            for w in range(n_win):
                v.wait_ge(gsems[w % g_bufs], lane_after[w])
                if w >= o_bufs:
                    wp = w - o_bufs
                    v.wait_ge(ssems[wp % o_bufs], 16 * (wp // o_bufs + 1))
                v.reduce_sum(
                    oslot(w),
                    gslot(w).rearrange("p (l e) -> p e l", e=d),
                    axis=mybir.AxisListType.X,
                ).then_inc(rsem, 1)

    return nc


def _plan(indices, offsets, pad_row):
    """Host-side planning: per-table padded bag-major index buffers.

    pad_row: index of the appended all-zeros row (= original table row count).
    Returns (idxbufs [T, P, sum_l] int32, lws, col_ofs, sum_l, need_pad).
    """
    idx64 = np.ascontiguousarray(indices).astype(np.int64)
    off = np.ascontiguousarray(offsets).astype(np.int64)
    t, total = idx64.shape
    b = off.shape[1]
    n_win = b // P

    ends = np.concatenate([off[:, 1:], np.full((t, 1), total, np.int64)], axis=1)
    lens = np.clip(ends - off, 0, None)  # [T, B]

    l_uniform = total // b
    fixed = (
        total == b * l_uniform
        and (lens == l_uniform).all()
        and (off == np.arange(b, dtype=np.int64) * l_uniform).all()
    )

    if fixed:
        lws = [l_uniform] * n_win
        col_ofs = [w * l_uniform for w in range(n_win)]
        sum_l = n_win * l_uniform
        # [t, b, l] -> [t, p, w*L+l]
        bufs = (
            idx64.reshape(t, n_win, P, l_uniform)
            .transpose(0, 2, 1, 3)
            .reshape(t, P, sum_l)
            .astype(np.int32)
        )
        return bufs, lws, col_ofs, sum_l, False

    lws = []
    col_ofs = []
    blocks = []
    need_pad = False
    for w in range(n_win):
        b0 = w * P
        lens_w = lens[:, b0 : b0 + P]  # [T, P]
        lw = max(1, int(lens_w.max()))
        if (lens_w != lw).any():
            need_pad = True
        l_grid = np.arange(lw, dtype=np.int64)
        pos = off[:, b0 : b0 + P, None] + l_grid[None, None, :]  # [T, P, lw]
        valid = l_grid[None, None, :] < lens_w[:, :, None]
        gathered = np.take_along_axis(
            idx64, pos.clip(0, total - 1).reshape(t, -1), axis=1
        ).reshape(t, P, lw)
        blocks.append(np.where(valid, gathered, pad_row).astype(np.int32))
        col_ofs.append(sum(lws))
        lws.append(lw)
    sum_l = sum(lws)
    bufs = np.concatenate(blocks, axis=2)
    return bufs, lws, col_ofs, sum_l, need_pad


def _plan2(indices, offsets, n_rows, chunk=25000):
    """Host planning for the dma_gather path.

    Rows of each 128-bag window are stable-sorted by table chunk
    (idx // chunk) so each run's local indices fit int16.  Runs are padded
    to a multiple of 16 (shared across tables) with dummy index 0; dummy /
    stale positions carry seg = -1 so the one-hot pooling drops them.

    Returns dict with per-table device buffers and the static schedule.
    """
    idx64 = np.ascontiguousarray(indices).astype(np.int64)
    off = np.ascontiguousarray(offsets).astype(np.int64)
    t, total = idx64.shape
    b = off.shape[1]
    n_win = b // P
    n_chunks = -(-n_rows // chunk)
    assert chunk <= 32767

    ends = np.concatenate([off[:, 1:], np.full((t, 1), total, np.int64)], axis=1)
    lens = np.clip(ends - off, 0, None)  # [T, B]

    # Per window, per table: positions and their bag (seg) in window-local
    # terms, sorted by chunk.
    idx_cols = []   # per-(w,c) int16 [T, P16] local indices
    seg_cols = []   # per-(w,slot) f32 [T, 128] segs
    sched = []      # per window: list of (chunk_id, P16, n_slots)
    l_uni = total // b
    uniform = (
        total == b * l_uni
        and (lens == l_uni).all()
        and (off == np.arange(b, dtype=np.int64) * l_uni).all()
    )
    seg_uni = np.repeat(np.arange(P), l_uni)

    for w in range(n_win):
        b0 = w * P
        per_table = []  # (idx_sorted, seg_sorted, chunk_sorted) per table
        for i in range(t):
            if uniform:
                ix = idx64[i, b0 * l_uni : (b0 + P) * l_uni]
                segs = seg_uni
            else:
                ls = lens[i, b0 : b0 + P]
                segs = np.repeat(np.arange(P), ls)
                pos = np.concatenate(
                    [
                        np.arange(off[i, b0 + j], off[i, b0 + j] + ls[j])
                        for j in range(P)
                    ]
                ) if ls.sum() else np.zeros(0, np.int64)
                ix = idx64[i, pos] if len(pos) else np.zeros(0, np.int64)
            c = ix // chunk
            order = np.argsort(c, kind="stable")
            per_table.append((ix[order], segs[order], c[order]))
        wsched = []
        for c in range(n_chunks):
            ns = [int((pt[2] == c).sum()) for pt in per_table]
            mx = max(ns)
            if mx == 0:
                continue
            p16 = -(-mx // 16) * 16
            n_slots = -(-p16 // P)
            ib = np.zeros((t, p16), np.int16)
            sb = np.full((t, n_slots * P), -1.0, np.float32)
            for i in range(t):
                sel = per_table[i][2] == c
                k = ns[i]
                ib[i, :k] = (per_table[i][0][sel] - c * chunk).astype(np.int16)
                sb[i, :k] = per_table[i][1][sel].astype(np.float32)
            idx_cols.append(ib)
            seg_cols.append(sb)
            wsched.append((c, p16, n_slots))
        if not wsched:
            # Empty window: one dummy run so the psum still gets written
            # (with zeros) before the copy-out.
            idx_cols.append(np.zeros((t, 16), np.int16))
            seg_cols.append(np.full((t, P), -1.0, np.float32))
            wsched.append((0, 16, 1))
        sched.append(wsched)

    # Device idx buffer: wrapped [16, cols] replicated to 128 partitions.
    iparts = []
    for ib in idx_cols:
        t_, p16 = ib.shape
        iparts.append(ib.reshape(t_, p16 // 16, 16).transpose(0, 2, 1))
    idxbuf16 = np.concatenate(iparts, axis=2)  # [T, 16, IC]
    idxbuf = np.tile(idxbuf16, (1, 8, 1))  # [T, 128, IC]
    # Device seg buffer: [T, 128, n_slots_total] (seg of (partition, slot)).
    sparts = [sb.reshape(t, -1, P).transpose(0, 2, 1) for sb in seg_cols]
    segbuf = np.concatenate(sparts, axis=2)
    iota = np.tile(np.arange(P, dtype=np.float32)[None, :], (P, 1))
    return {
        "sched": sched,
        "idxbuf": np.ascontiguousarray(idxbuf),
        "segbuf": np.ascontiguousarray(segbuf),
        "iota": iota,
        "chunk": chunk,
    }


def _build_program2(n_rows, d, plan, g_bufs=4, oh_bufs=6, p_bufs=2, o_bufs=4):
    """dma_gather + one-hot-matmul pooling program (raw Bass)."""
    import contextlib

    import concourse.bass as bass
    import concourse.mybir as mybir
    from concourse import library_config

    sched = plan["sched"]
    chunk = plan["chunk"]
    n_win = len(sched)
    ic = plan["idxbuf"].shape[2]
    sc = plan["segbuf"].shape[2]

    # Static per-window derived counts.
    slots_per_win = [sum(ns for _, _, ns in ws) for ws in sched]
    g_per_win = [len(ws) for ws in sched]
    smax = max(slots_per_win)
    mm_after = np.cumsum(slots_per_win)  # matmuls (= slots) completed after w
    lane_after = {}
    lane_tot = [0] * g_bufs
    for w in range(n_win):
        lane_tot[w % g_bufs] += 16 * g_per_win[w]
        lane_after[w] = lane_tot[w % g_bufs]

    nc = bass.Bass(num_swdge_queues=4)
    wz = nc.declare_dram_parameter("wz", [n_rows, d], mybir.dt.float32, isOutput=False)
    idx = nc.declare_dram_parameter("idx", [P, ic], mybir.dt.int16, isOutput=False)
    seg = nc.declare_dram_parameter("seg", [P, sc], mybir.dt.float32, isOutput=False)
    iota = nc.declare_dram_parameter("iota", [P, P], mybir.dt.float32, isOutput=False)
    out = nc.declare_dram_parameter(
        "out", [n_win * P, d], mybir.dt.float32, isOutput=True
    )

    with contextlib.ExitStack() as ctx:
        idx_sb = ctx.enter_context(nc.sbuf_tensor([P, ic], mybir.dt.int16))
        seg_sb = ctx.enter_context(nc.sbuf_tensor([P, sc], mybir.dt.float32))
        iota_sb = ctx.enter_context(nc.sbuf_tensor([P, P], mybir.dt.float32))
        gbuf = ctx.enter_context(
            nc.sbuf_tensor([P, g_bufs * smax * d], mybir.dt.float32)
        )
        ohbuf = ctx.enter_context(nc.sbuf_tensor([P, oh_bufs * P], mybir.dt.float32))
        obuf = ctx.enter_context(nc.sbuf_tensor([P, o_bufs * d], mybir.dt.float32))
        psums = [
            ctx.enter_context(nc.psum_tensor(f"ps{i}", [P, d], mybir.dt.float32))
            for i in range(p_bufs)
        ]
        in_sem = ctx.enter_context(nc.semaphore("in_sem"))
        zsem = ctx.enter_context(nc.semaphore("zsem"))
        gsems = [ctx.enter_context(nc.semaphore(f"gsem{i}")) for i in range(g_bufs)]
        ohsem = ctx.enter_context(nc.semaphore("ohsem"))
        mmsem = ctx.enter_context(nc.semaphore("mmsem"))
        csem = ctx.enter_context(nc.semaphore("csem"))
        ssems = [ctx.enter_context(nc.semaphore(f"ssem{i}")) for i in range(o_bufs)]
        block = ctx.enter_context(nc.Block())

        @block.sync
        def _(sync):
            sync.dma_start(idx_sb[:], idx[:]).then_inc(in_sem, 16)
            sync.dma_start(seg_sb[:], seg[:]).then_inc(in_sem, 16)
            sync.dma_start(iota_sb[:], iota[:]).then_inc(in_sem, 16)
            for w in range(n_win):
                sync.wait_ge(csem, w + 1)
                sync.dma_start(
                    out[w * P : (w + 1) * P, :],
                    obuf[:, (w % o_bufs) * d : (w % o_bufs + 1) * d],
                ).then_inc(ssems[w % o_bufs], 16)
            for lane in range(o_bufs):
                n_l = len(range(lane, n_win, o_bufs))
                if n_l:
                    sync.wait_ge(ssems[lane], 16 * n_l)

        @block.gpsimd
        def _(g):
            g.load_library(library_config.mlp)
            # First-ever use of gbuf: ensure finite contents so one-hot
            # zero-columns can't turn stale NaNs into NaN outputs.
            g.memset(gbuf[:], 0.0).then_inc(zsem, 1)
            g.wait_ge(zsem, 1)
            g.wait_ge(in_sem, 48)
            reg_ctx = g.register("ni_reg")
            ni = reg_ctx.__enter__()
            icol = 0
            for w in range(n_win):
                if w >= g_bufs:
                    g.wait_ge(mmsem, int(mm_after[w - g_bufs]))
                base = (w % g_bufs) * (smax * d)
                sofs = 0
                for c, p16, n_slots in sched[w]:
                    g.reg_mov(ni, p16)
                    g.dma_gather(
                        out_ap=gbuf[
                            :, base + sofs * d : base + (sofs + n_slots) * d
                        ].rearrange("p (s e) -> p s e", e=d),
                        in_ap=wz[c * chunk : min((c + 1) * chunk, n_rows), :],
                        idxs_ap=idx_sb[:, icol : icol + p16 // 16],
                        num_idxs=p16,
                        num_idxs_reg=ni,
                        elem_size=d,
                        single_packet=False,
                        queue_num=w % g_bufs % 4,
                    ).then_inc(gsems[w % g_bufs], 16)
                    icol += p16 // 16
                    sofs += n_slots

        @block.vector
        def _(v):
            v.wait_ge(in_sem, 48)
            j = 0  # global slot index
            for w in range(n_win):
                for s in range(slots_per_win[w]):
                    if j >= oh_bufs:
                        v.wait_ge(mmsem, j - oh_bufs + 1)
                    v.tensor_tensor(
                        out=ohbuf[:, (j % oh_bufs) * P : (j % oh_bufs + 1) * P],
                        in0=seg_sb[:, j : j + 1].to_broadcast([P, P]),
                        in1=iota_sb[:],
                        op=mybir.AluOpType.is_equal,
                    ).then_inc(ohsem, 1)
                    j += 1

        @block.tensor
        def _(pe):
            pe.wait_ge(zsem, 1)
            j = 0
            for w in range(n_win):
                base = (w % g_bufs) * (smax * d)
                pe.wait_ge(gsems[w % g_bufs], lane_after[w])
                if w >= p_bufs:
                    pe.wait_ge(csem, w - p_bufs + 1)
                ns = slots_per_win[w]
                for s in range(ns):
                    pe.wait_ge(ohsem, j + 1)
                    pe.matmul(
                        psums[w % p_bufs][:],
                        lhsT=ohbuf[:, (j % oh_bufs) * P : (j % oh_bufs + 1) * P],
                        rhs=gbuf[:, base + s * d : base + (s + 1) * d],
                        start=(s == 0),
                        stop=(s == ns - 1),
                    ).then_inc(mmsem, 1)
                    j += 1

        @block.scalar
        def _(a):
            for w in range(n_win):
                a.wait_ge(mmsem, int(mm_after[w]))
                if w >= o_bufs:
                    wp = w - o_bufs
                    a.wait_ge(ssems[wp % o_bufs], 16 * (wp // o_bufs + 1))
                a.copy(
                    obuf[:, (w % o_bufs) * d : (w % o_bufs + 1) * d],
                    psums[w % p_bufs][:],
                ).then_inc(csem, 1)

    return nc


def _run(weights, indices, offsets, trace=False, v2=True, chunk=None):
    from concourse import mybir
    from concourse.bass_utils import run_bass_kernel_spmd

    weights = np.ascontiguousarray(np.asarray(weights), dtype=np.float32)
    t, n, d = weights.shape

    if v2:
        if chunk is None:
            chunk = -(-n // max(1, -(-n // 32767)))  # even chunks, each <= 32767
        plan = _plan2(indices, offsets, n, chunk=chunk)
        nc = _build_program2(n, d, plan)
        mybir.codegen_inst_isa_subclasses(nc)
        in_maps = [
            {
                "wz": weights[i],
                "idx": np.ascontiguousarray(plan["idxbuf"][i]),
                "seg": np.ascontiguousarray(plan["segbuf"][i]),
                "iota": plan["iota"],
            }
            for i in range(t)
        ]
    else:
        idxbufs, lws, col_ofs, sum_l, need_pad = _plan(indices, offsets, n)
        n_win = np.asarray(offsets).shape[1] // P
        if need_pad:
            wz = np.concatenate([weights, np.zeros((t, 1, d), np.float32)], axis=1)
        else:
            wz = weights
        nc = _build_program(wz.shape[1], d, n_win, lws, col_ofs, sum_l)
        in_maps = [
            {"wz": wz[i], "idx": np.ascontiguousarray(idxbufs[i])} for i in range(t)
        ]
    res = run_bass_kernel_spmd(nc, in_maps, list(range(t)), trace=trace)
    out = np.stack([res.results[i]["out"] for i in range(t)], axis=0)
    return out, res


def kernel(weights, indices, offsets):
    out, _ = _run(weights, indices, offsets)
    return out



# revision 16
# speedup vs baseline: 1.7362x; 1.7362x over previous
"""MergedEmbeddingBag forward (sum pooling) on 8 Trainium2 NeuronCores.

Strategy (table-parallel, per sharding hint): core t owns table t.

v3 pipeline per core:
  - weights are cast to bf16 on the host and uploaded as [N, 128] bf16, so
    each gathered row is 256 B (half the HBM traffic of fp32) and feeds
    1-pass bf16 matmuls.
  - indices are planned on the host into supergroups of G windows (a window
    = 128 bags); within a supergroup, items are split by table chunk
    (N <= 4 chunks of <= 32767 rows so local indices fit signed int16) and
    gathered with one dma_gather per (supergroup, chunk) - few large SWDGE
    calls instead of many small ones.  Trailing pads use index -1 (skipped
    by HW, no traffic).
  - pooling: for each 128-row slot of gathered data, a one-hot bf16 matrix
    (generated on DVE with a single batched is_equal per call) scatters the
    rows into their bags via a PE matmul accumulated in PSUM.  Slots that
    span window boundaries are matmul'd once per window with masked one-hot
    columns, so there is no per-window padding of the gather stream.
  - scalar (ACT) copies finished PSUM windows to SBUF; sync (HWDGE) stores
    them to HBM.
"""

import sys

sys.path.insert(0, "/opt/trn_rl_repo")

import numpy as np

P = 128  # partitions / bags per window
D = 128  # embedding dim


def _plan3(indices, offsets, n_rows, G=4, max_chunk=32767):
    """Host planning: supergrouped, chunk-split, window-sorted gather order.

    Returns dict with device buffers (idxbuf int16, segbuf bf16 as uint16
    view, iota) and the static schedule consumed by _build_program3.
    """
    import ml_dtypes

    idx64 = np.ascontiguousarray(indices).astype(np.int64)
    off = np.ascontiguousarray(offsets).astype(np.int64)
    t, total = idx64.shape
    b = off.shape[1]
    assert b % P == 0
    n_win = b // P
    assert n_win % G == 0
    n_groups = n_win // G
    n_chunks = -(-n_rows // max_chunk)
    chunk = -(-n_rows // n_chunks)
    assert chunk <= max_chunk

    ends = np.concatenate([off[:, 1:], np.full((t, 1), total, np.int64)], axis=1)
    lens = np.clip(ends - off, 0, None)  # [T, B]

    l_uni = total // b
    uniform = (
        total == b * l_uni
        and (lens == l_uni).all()
        and (off == np.arange(b, dtype=np.int64) * l_uni).all()
    )

    # Per-table flat item lists with window-local bag and window ids, in
    # (window, bag, position) order.
    plans = []
    for i in range(t):
        if uniform:
            vals = idx64[i]  # already (bag, l) order; bags in window order
            bag = (np.arange(total) // l_uni) % P
            win = np.arange(total) // (P * l_uni)
        else:
            ls = lens[i]
            bag_of_item = np.repeat(np.arange(b), ls)
            pos = np.concatenate(
                [np.arange(off[i, j], off[i, j] + ls[j]) for j in range(b)]
            ) if ls.sum() else np.zeros(0, np.int64)
            vals = idx64[i, pos]
            bag = bag_of_item % P
            win = bag_of_item // P
        plans.append((vals, bag, win))

    # Shared static schedule across tables: per (group, chunk) call sizes
    # must be identical for the single SPMD program, so take the max and
    # pad with -1 (skipped).  mm schedule must also be shared: a matmul
    # exists for (call, slot, window) if ANY table has items there; its
    # seg column is per-table data.
    calls = []  # static: per call dict
    idx_cols = []  # per call: [T, p16] int16 (pad -1)
    seg_cols = []  # per call: [T, n_mm, 128] float (pad -1)
    for g in range(n_groups):
        w0, w1 = g * G, (g + 1) * G
        for c in range(n_chunks):
            per_t = []
            for i in range(t):
                vals, bag, win = plans[i]
                m = (win >= w0) & (win < w1) & (vals >= c * chunk) & (
                    vals < (c + 1) * chunk
                )
                per_t.append(
                    (
                        (vals[m] - c * chunk).astype(np.int16),
                        bag[m].astype(np.int16),
                        (win[m] - w0).astype(np.int16),
                    )
                )
            nmax = max(len(v) for v, _, _ in per_t)
            nmax = max(nmax, 1)
            p16 = -(-nmax // 16) * 16
            n_slots = -(-p16 // P)
            iv = np.full((t, p16), -1, np.int16)
            bv = np.full((t, n_slots * P), -1, np.int16)
            wv = np.full((t, n_slots * P), -2, np.int16)
            nvalid = np.zeros(t, np.int64)
            for i in range(t):
                v, bg, wn = per_t[i]
                k = len(v)
                iv[i, :k] = v
                bv[i, :k] = bg
                wv[i, :k] = wn
                nvalid[i] = k
            # mm list: per slot, windows present in ANY table (ascending)
            mms = []
            segs = []
            for s in range(n_slots):
                sl = slice(s * P, (s + 1) * P)
                wins_here = np.unique(wv[:, sl])
                wins_here = wins_here[wins_here >= 0]
                for wloc in wins_here:
                    seg = np.where(wv[:, sl] == wloc, bv[:, sl], -1).astype(
                        np.float32
                    )  # [T, 128]
                    mms.append((s, int(wloc)))
                    segs.append(seg)
            calls.append(
                dict(
                    group=g,
                    chunk=c,
                    p16=p16,
                    n_slots=n_slots,
                    nvalid=nvalid,
                    mms=mms,
                )
            )
            idx_cols.append(iv)
            seg_cols.append(
                np.stack(segs, axis=1) if segs else np.zeros((t, 0, P), np.float32)
            )

    # start/stop flags per matmul: per group, first/last mm of each window.
    n_per_group = len(calls) // n_groups
    mm_global = 0
    for g in range(n_groups):
        seen = {}
        order = []  # (call_idx, mm_idx, wloc)
        for cc in range(g * n_per_group, (g + 1) * n_per_group):
            for mi, (s, wloc) in enumerate(calls[cc]["mms"]):
                order.append((cc, mi, wloc))
        firsts, lasts = {}, {}
        for k, (cc, mi, wloc) in enumerate(order):
            if wloc not in firsts:
                firsts[wloc] = (cc, mi)
            lasts[wloc] = (cc, mi)
        for cc in range(g * n_per_group, (g + 1) * n_per_group):
            flags = []
            for mi, (s, wloc) in enumerate(calls[cc]["mms"]):
                flags.append(
                    (
                        firsts[wloc] == (cc, mi),
                        lasts[wloc] == (cc, mi),
                    )
                )
            calls[cc]["flags"] = flags
        # every window in the group must have at least one mm (else its
        # psum region is never written); guaranteed here because every
        # window has >= 1 item in >= 1 chunk.  Guard anyway:
        assert len(firsts) == G or b == 0, (g, sorted(firsts))
        mm_global += len(order)

    # Device buffers.
    # idxbuf: concat per-call [p16] wrapped to [16, p16/16], tiled to 128.
    iparts = []
    for iv in idx_cols:
        p16 = iv.shape[1]
        iparts.append(iv.reshape(t, p16 // 16, 16).transpose(0, 2, 1))
    idxbuf16 = np.concatenate(iparts, axis=2)  # [T, 16, IC]
    idxbuf = np.ascontiguousarray(np.tile(idxbuf16, (1, 8, 1)))  # [T, 128, IC]

    # segbuf: [T, 128, M_total] bf16 (partition p = slot row)
    sparts = [sc.transpose(0, 2, 1) for sc in seg_cols]  # [T, 128, n_mm]
    segbuf = np.concatenate(sparts, axis=2).astype(ml_dtypes.bfloat16)

    iota = np.tile(
        np.arange(P, dtype=np.float32)[None, :], (P, 1)
    ).astype(ml_dtypes.bfloat16)

    return dict(
        calls=calls,
        idxbuf=idxbuf,
        segbuf=np.ascontiguousarray(segbuf),
        iota=iota,
        chunk=chunk,
        n_chunks=n_chunks,
        G=G,
        n_groups=n_groups,
        n_win=n_win,
    )


def _build_program3(n_rows, plan, nbuf=3, ohb=6, o_bufs=4, scratch=16384):
    """Raw-Bass SPMD program for the v3 pipeline."""
    import contextlib

    import concourse.bass as bass
    import concourse.mybir as mybir
    from concourse import library_config

    calls = plan["calls"]
    chunk = plan["chunk"]
    G = plan["G"]
    n_groups = plan["n_groups"]
    n_win = plan["n_win"]
    n_calls = len(calls)
    n_per_group = n_calls // n_groups
    ic = plan["idxbuf"].shape[2]
    sc = max(1, plan["segbuf"].shape[2])

    # static derived counts
    call_mm = [len(c["mms"]) for c in calls]
    mm_cum = np.cumsum([0] + call_mm)  # mm count before call j
    group_slots = [
        sum(calls[cc]["n_slots"] for cc in range(g * n_per_group, (g + 1) * n_per_group))
        for g in range(n_groups)
    ]
    region_slots = max(group_slots)
    call_mm_max = max(call_mm)
    # group gbuf slot offsets per call
    call_goff = []
    for g in range(n_groups):
        o = 0
        for cc in range(g * n_per_group, (g + 1) * n_per_group):
            call_goff.append(o)
            o += calls[cc]["n_slots"]
    # per-queue cumulative call counts
    q_of_call = [j % 4 for j in range(n_calls)]
    q_count_after = []
    qc = [0, 0, 0, 0]
    for j in range(n_calls):
        qc[q_of_call[j]] += 1
        q_count_after.append(qc[q_of_call[j]])
    # mm index of each window's last matmul (global, in issue order)
    last_mm_of_win = {}
    first_mm_of_win = {}
    k = 0
    for j, c in enumerate(calls):
        g = c["group"]
        for (s, wloc) in c["mms"]:
            w = g * G + wloc
            if w not in first_mm_of_win:
                first_mm_of_win[w] = k
            last_mm_of_win[w] = k
            k += 1
    mm_total = k

    bf16 = mybir.dt.bfloat16
    f32 = mybir.dt.float32

    nc = bass.Bass(num_swdge_queues=4, dynamic_dma_scratch_size=scratch)
    wz = nc.declare_dram_parameter("wz", [n_rows, D], bf16, isOutput=False)
    idx = nc.declare_dram_parameter("idx", [P, ic], mybir.dt.int16, isOutput=False)
    seg = nc.declare_dram_parameter("seg", [P, sc], bf16, isOutput=False)
    iota = nc.declare_dram_parameter("iota", [P, P], bf16, isOutput=False)
    out = nc.declare_dram_parameter("out", [n_win * P, D], f32, isOutput=True)

    with contextlib.ExitStack() as ctx:
        idx_sb = ctx.enter_context(nc.sbuf_tensor([P, ic], mybir.dt.int16))
        seg_sb = ctx.enter_context(nc.sbuf_tensor([P, sc], bf16))
        iota_sb = ctx.enter_context(nc.sbuf_tensor([P, P], bf16))
        gbuf = ctx.enter_context(
            nc.sbuf_tensor([P, nbuf * region_slots * D], bf16)
        )
        ohbuf = ctx.enter_context(
            nc.sbuf_tensor([P, ohb * call_mm_max * P], bf16)
        )
        obuf = ctx.enter_context(nc.sbuf_tensor([P, o_bufs * D], f32))
        # One PSUM BANK per in-flight window: a matmul's start=True resets
        # the whole bank, so windows must not share banks while accumulating.
        assert G <= 4
        psums = [
            ctx.enter_context(nc.psum_tensor(f"ps{i}", [P, P], f32))
            for i in range(2 * G)
        ]
        in_sem = ctx.enter_context(nc.semaphore("in_sem"))
        zsem = ctx.enter_context(nc.semaphore("zsem"))
        # One gather-completion sem per (region, call-in-group) so at most
        # ONE DMA is ever in flight per sem (per-engine increments from two
        # concurrent DMAs interleave, so cumulative ge-16k waits on a shared
        # sem are racy).  Region gating (mmsem) bounds in-flight per sem to 1.
        n_gsem = nbuf * n_per_group
        gsems = [ctx.enter_context(nc.semaphore(f"gsem{i}")) for i in range(n_gsem)]
        ohsem = ctx.enter_context(nc.semaphore("ohsem"))
        mmsem = ctx.enter_context(nc.semaphore("mmsem"))
        csem = ctx.enter_context(nc.semaphore("csem"))
        ssems = [ctx.enter_context(nc.semaphore(f"ssem{i}")) for i in range(o_bufs)]
        block = ctx.enter_context(nc.Block())

        def gslot(j, s):
            """SBUF tile [P, D] of slot s of call j."""
            g = calls[j]["group"]
            base = (g % nbuf) * region_slots * D + (call_goff[j] + s) * D
            return gbuf[:, base : base + D]

        def gdest(j):
            g = calls[j]["group"]
            base = (g % nbuf) * region_slots * D + call_goff[j] * D
            n_slots = calls[j]["n_slots"]
            return gbuf[:, base : base + n_slots * D].rearrange(
                "p (s e) -> p s e", e=D
            )

        def ohcol(m_global, j):
            r = j % ohb
            off = (m_global - mm_cum[j]) * P
            return ohbuf[:, r * call_mm_max * P + off : r * call_mm_max * P + off + P]

        @block.sync
        def _(sync):
            sync.dma_start(idx_sb[:], idx[:]).then_inc(in_sem, 16)
            sync.dma_start(seg_sb[:], seg[:]).then_inc(in_sem, 16)
            sync.dma_start(iota_sb[:], iota[:]).then_inc(in_sem, 16)
            for w in range(n_win):
                sync.wait_ge(csem, w + 1)
                sync.dma_start(
                    out[w * P : (w + 1) * P, :],
                    obuf[:, (w % o_bufs) * D : (w % o_bufs + 1) * D],
                ).then_inc(ssems[w % o_bufs], 16)
            for lane in range(o_bufs):
                n_l = len(range(lane, n_win, o_bufs))
                if n_l:
                    sync.wait_ge(ssems[lane], 16 * n_l)

        @block.gpsimd
        def _(g):
            g.load_library(library_config.mlp)
            g.memset(gbuf[:], 0.0).then_inc(zsem, 1)
            g.wait_ge(zsem, 1)
            g.wait_ge(in_sem, 48)
            reg_ctx = g.register("ni_reg")
            ni = reg_ctx.__enter__()
            icol = 0
            for j, c in enumerate(calls):
                grp = c["group"]
                if j % n_per_group == 0 and grp >= nbuf:
                    g.wait_ge(mmsem, int(mm_cum[(grp - nbuf + 1) * n_per_group]))
                # nvalid differs per table but the SPMD program is shared,
                # so pads point at row 0 (valid) and every core gathers p16.
                g.reg_mov(ni, c["p16"])
                g.dma_gather(
                    out_ap=gdest(j),
                    in_ap=wz[c["chunk"] * chunk : min((c["chunk"] + 1) * chunk, n_rows), :],
                    idxs_ap=idx_sb[:, icol : icol + c["p16"] // 16],
                    num_idxs=c["p16"],
                    num_idxs_reg=ni,
                    elem_size=D,
                    single_packet=False,
                    queue_num=q_of_call[j],
                ).then_inc(gsems[j % n_gsem], 16)
                icol += c["p16"] // 16

        @block.vector
        def _(v):
            v.wait_ge(in_sem, 48)
            for j, c in enumerate(calls):
                n_mm = call_mm[j]
                if n_mm == 0:
                    continue
                if j >= ohb:
                    v.wait_ge(mmsem, int(mm_cum[j - ohb + 1]))
                r = j % ohb
                o = ohbuf[
                    :, r * call_mm_max * P : r * call_mm_max * P + n_mm * P
                ].rearrange("p (m e) -> p m e", e=P)
                s_in = (
                    seg_sb[:, mm_cum[j] : mm_cum[j + 1]]
                    .rearrange("p (m o) -> p m o", o=1)
                    .broadcast_to([P, n_mm, P])
                )
                i_in = (
                    iota_sb[:]
                    .rearrange("p (o e) -> p o e", o=1)
                    .broadcast_to([P, n_mm, P])
                )
                v.tensor_tensor(
                    out=o, in0=s_in, in1=i_in, op=mybir.AluOpType.is_equal
                ).then_inc(ohsem, 1)

        @block.tensor
        def _(pe):
            m_global = 0
            for j, c in enumerate(calls):
                grp = c["group"]
                if j % n_per_group == 0 and grp >= 2:
                    # psum region (grp % 2) free when group grp-2 fully copied
                    pe.wait_ge(csem, (grp - 1) * G)
                pe.wait_ge(gsems[j % n_gsem], 16 * (j // n_gsem + 1))
                if call_mm[j]:
                    pe.wait_ge(ohsem, sum(1 for jj in range(j + 1) if call_mm[jj]))
                for mi, (s, wloc) in enumerate(c["mms"]):
                    st, sp = c["flags"][mi]
                    pe.matmul(
                        psums[(grp % 2) * G + wloc][:],
                        lhsT=ohcol(m_global, j),
                        rhs=gslot(j, s),
                        start=st,
                        stop=sp,
                        skip_group_check=True,
                    ).then_inc(mmsem, 1)
                    m_global += 1

        @block.scalar
        def _(a):
            for w in range(n_win):
                a.wait_ge(mmsem, int(last_mm_of_win[w]) + 1)
                if w >= o_bufs:
                    wp = w - o_bufs
                    a.wait_ge(ssems[wp % o_bufs], 16 * (wp // o_bufs + 1))
                grp = w // G
                wloc = w % G
                a.copy(
                    obuf[:, (w % o_bufs) * D : (w % o_bufs + 1) * D],
                    psums[(grp % 2) * G + wloc][:],
                ).then_inc(csem, 1)

    return nc


def _run(weights, indices, offsets, trace=False, G=4, scratch=16384):
    import ml_dtypes
    from concourse import mybir
    from concourse.bass_utils import run_bass_kernel_spmd

    weights = np.asarray(weights)
    t, n, d = weights.shape
    assert d == D

    plan = _plan3(indices, offsets, n, G=G)

    # pads must gather a real row (see note in gpsimd block): rewrite -1
    # pads in idxbuf to 0.
    idxbuf = plan["idxbuf"].copy()
    idxbuf[idxbuf < 0] = 0

    wz16 = weights.astype(ml_dtypes.bfloat16)

    nc = _build_program3(n, plan, scratch=scratch)
    mybir.codegen_inst_isa_subclasses(nc)
    in_maps = [
        {
            "wz": np.ascontiguousarray(wz16[i]),
            "idx": np.ascontiguousarray(idxbuf[i]),
            "seg": np.ascontiguousarray(plan["segbuf"][i]),
            "iota": plan["iota"],
        }
        for i in range(t)
    ]
    res = run_bass_kernel_spmd(nc, in_maps, list(range(t)), trace=trace)
    out = np.stack([res.results[i]["out"] for i in range(t)], axis=0)
    return out, res


def kernel(weights, indices, offsets):
    out, _ = _run(weights, indices, offsets)
    return out


# revision 19
# speedup vs baseline: 1.7718x; 1.0205x over previous
"""MergedEmbeddingBag forward (sum pooling) on 8 Trainium2 NeuronCores.

Strategy (table-parallel, per sharding hint): core t owns table t.

v3 pipeline per core:
  - weights are cast to bf16 on the host and uploaded as [N, 128] bf16, so
    each gathered row is 256 B (half the HBM traffic of fp32) and feeds
    1-pass bf16 matmuls.
  - indices are planned on the host into supergroups of G windows (a window
    = 128 bags); within a supergroup, items are split by table chunk
    (N <= 4 chunks of <= 32767 rows so local indices fit signed int16) and
    gathered with one dma_gather per (supergroup, chunk) - few large SWDGE
    calls instead of many small ones.  Trailing pads use index -1 (skipped
    by HW, no traffic).
  - pooling: for each 128-row slot of gathered data, a one-hot bf16 matrix
    (generated on DVE with a single batched is_equal per call) scatters the
    rows into their bags via a PE matmul accumulated in PSUM.  Slots that
    span window boundaries are matmul'd once per window with masked one-hot
    columns, so there is no per-window padding of the gather stream.
  - scalar (ACT) copies finished PSUM windows to SBUF; sync (HWDGE) stores
    them to HBM.
"""

import sys

sys.path.insert(0, "/opt/trn_rl_repo")

import numpy as np

P = 128  # partitions / bags per window
D = 128  # embedding dim


def _plan3(indices, offsets, n_rows, G=4, max_chunk=32767):
    """Host planning: supergrouped, chunk-split, window-sorted gather order.

    Returns dict with device buffers (idxbuf int16, segbuf bf16 as uint16
    view, iota) and the static schedule consumed by _build_program3.
    """
    import ml_dtypes

    idx64 = np.ascontiguousarray(indices).astype(np.int64)
    off = np.ascontiguousarray(offsets).astype(np.int64)
    t, total = idx64.shape
    b = off.shape[1]
    assert b % P == 0
    n_win = b // P
    assert n_win % G == 0
    n_groups = n_win // G
    n_chunks = -(-n_rows // max_chunk)
    chunk = -(-n_rows // n_chunks)
    assert chunk <= max_chunk

    ends = np.concatenate([off[:, 1:], np.full((t, 1), total, np.int64)], axis=1)
    lens = np.clip(ends - off, 0, None)  # [T, B]

    l_uni = total // b
    uniform = (
        total == b * l_uni
        and (lens == l_uni).all()
        and (off == np.arange(b, dtype=np.int64) * l_uni).all()
    )

    # Per-table flat item lists with window-local bag and window ids, in
    # (window, bag, position) order.
    plans = []
    for i in range(t):
        if uniform:
            vals = idx64[i]  # already (bag, l) order; bags in window order
            bag = (np.arange(total) // l_uni) % P
            win = np.arange(total) // (P * l_uni)
        else:
            ls = lens[i]
            bag_of_item = np.repeat(np.arange(b), ls)
            pos = np.concatenate(
                [np.arange(off[i, j], off[i, j] + ls[j]) for j in range(b)]
            ) if ls.sum() else np.zeros(0, np.int64)
            vals = idx64[i, pos]
            bag = bag_of_item % P
            win = bag_of_item // P
        plans.append((vals, bag, win))

    # Shared static schedule across tables: per (group, chunk) call sizes
    # must be identical for the single SPMD program, so take the max and
    # pad with -1 (skipped).  mm schedule must also be shared: a matmul
    # exists for (call, slot, window) if ANY table has items there; its
    # seg column is per-table data.
    calls = []  # static: per call dict
    idx_cols = []  # per call: [T, p16] int16 (pad -1)
    seg_cols = []  # per call: [T, n_mm, 128] float (pad -1)
    for g in range(n_groups):
        w0, w1 = g * G, (g + 1) * G
        for c in range(n_chunks):
            per_t = []
            for i in range(t):
                vals, bag, win = plans[i]
                m = (win >= w0) & (win < w1) & (vals >= c * chunk) & (
                    vals < (c + 1) * chunk
                )
                per_t.append(
                    (
                        (vals[m] - c * chunk).astype(np.int16),
                        bag[m].astype(np.int16),
                        (win[m] - w0).astype(np.int16),
                    )
                )
            nmax = max(len(v) for v, _, _ in per_t)
            nmax = max(nmax, 1)
            # pad to FULL slots (pads rewritten to row 0 later): every gbuf
            # byte that a matmul can read gets written by its own call, so no
            # stale-NaN risk and no startup memset is needed.
            n_slots = -(-nmax // P)
            p16 = n_slots * P
            iv = np.full((t, p16), -1, np.int16)
            bv = np.full((t, n_slots * P), -1, np.int16)
            wv = np.full((t, n_slots * P), -2, np.int16)
            nvalid = np.zeros(t, np.int64)
            for i in range(t):
                v, bg, wn = per_t[i]
                k = len(v)
                iv[i, :k] = v
                bv[i, :k] = bg
                wv[i, :k] = wn
                nvalid[i] = k
            # mm list: per slot, windows present in ANY table (ascending)
            mms = []
            segs = []
            for s in range(n_slots):
                sl = slice(s * P, (s + 1) * P)
                wins_here = np.unique(wv[:, sl])
                wins_here = wins_here[wins_here >= 0]
                for wloc in wins_here:
                    seg = np.where(wv[:, sl] == wloc, bv[:, sl], -1).astype(
                        np.float32
                    )  # [T, 128]
                    mms.append((s, int(wloc)))
                    segs.append(seg)
            calls.append(
                dict(
                    group=g,
                    chunk=c,
                    p16=p16,
                    n_slots=n_slots,
                    nvalid=nvalid,
                    mms=mms,
                )
            )
            idx_cols.append(iv)
            seg_cols.append(
                np.stack(segs, axis=1) if segs else np.zeros((t, 0, P), np.float32)
            )

    # start/stop flags per matmul: per group, first/last mm of each window.
    n_per_group = len(calls) // n_groups
    mm_global = 0
    for g in range(n_groups):
        seen = {}
        order = []  # (call_idx, mm_idx, wloc)
        for cc in range(g * n_per_group, (g + 1) * n_per_group):
            for mi, (s, wloc) in enumerate(calls[cc]["mms"]):
                order.append((cc, mi, wloc))
        firsts, lasts = {}, {}
        for k, (cc, mi, wloc) in enumerate(order):
            if wloc not in firsts:
                firsts[wloc] = (cc, mi)
            lasts[wloc] = (cc, mi)
        for cc in range(g * n_per_group, (g + 1) * n_per_group):
            flags = []
            for mi, (s, wloc) in enumerate(calls[cc]["mms"]):
                flags.append(
                    (
                        firsts[wloc] == (cc, mi),
                        lasts[wloc] == (cc, mi),
                    )
                )
            calls[cc]["flags"] = flags
        # every window in the group must have at least one mm (else its
        # psum region is never written); guaranteed here because every
        # window has >= 1 item in >= 1 chunk.  Guard anyway:
        assert len(firsts) == G or b == 0, (g, sorted(firsts))
        mm_global += len(order)

    # Device buffers.
    # idxbuf: concat per-call [p16] wrapped to [16, p16/16], tiled to 128.
    iparts = []
    for iv in idx_cols:
        p16 = iv.shape[1]
        iparts.append(iv.reshape(t, p16 // 16, 16).transpose(0, 2, 1))
    idxbuf16 = np.concatenate(iparts, axis=2)  # [T, 16, IC]
    idxbuf = np.ascontiguousarray(np.tile(idxbuf16, (1, 8, 1)))  # [T, 128, IC]

    # segbuf: [T, 128, M_total] bf16 (partition p = slot row)
    sparts = [sc.transpose(0, 2, 1) for sc in seg_cols]  # [T, 128, n_mm]
    segbuf = np.concatenate(sparts, axis=2).astype(ml_dtypes.bfloat16)

    iota = np.tile(
        np.arange(P, dtype=np.float32)[None, :], (P, 1)
    ).astype(ml_dtypes.bfloat16)

    return dict(
        calls=calls,
        idxbuf=idxbuf,
        segbuf=np.ascontiguousarray(segbuf),
        iota=iota,
        chunk=chunk,
        n_chunks=n_chunks,
        G=G,
        n_groups=n_groups,
        n_win=n_win,
    )


def _build_program3(n_rows, plan, nbuf=3, ohb=6, o_bufs=4, scratch=32768):
    """Raw-Bass SPMD program for the v3 pipeline."""
    import contextlib

    import concourse.bass as bass
    import concourse.mybir as mybir
    from concourse import library_config

    calls = plan["calls"]
    chunk = plan["chunk"]
    G = plan["G"]
    n_groups = plan["n_groups"]
    n_win = plan["n_win"]
    n_calls = len(calls)
    n_per_group = n_calls // n_groups
    ic = plan["idxbuf"].shape[2]
    sc = max(1, plan["segbuf"].shape[2])

    # static derived counts
    call_mm = [len(c["mms"]) for c in calls]
    mm_cum = np.cumsum([0] + call_mm)  # mm count before call j
    group_slots = [
        sum(calls[cc]["n_slots"] for cc in range(g * n_per_group, (g + 1) * n_per_group))
        for g in range(n_groups)
    ]
    region_slots = max(group_slots)
    call_mm_max = max(call_mm)
    # group gbuf slot offsets per call
    call_goff = []
    for g in range(n_groups):
        o = 0
        for cc in range(g * n_per_group, (g + 1) * n_per_group):
            call_goff.append(o)
            o += calls[cc]["n_slots"]
    # per-queue cumulative call counts
    q_of_call = [j % 4 for j in range(n_calls)]
    q_count_after = []
    qc = [0, 0, 0, 0]
    for j in range(n_calls):
        qc[q_of_call[j]] += 1
        q_count_after.append(qc[q_of_call[j]])
    # mm index of each window's last matmul (global, in issue order)
    last_mm_of_win = {}
    first_mm_of_win = {}
    k = 0
    for j, c in enumerate(calls):
        g = c["group"]
        for (s, wloc) in c["mms"]:
            w = g * G + wloc
            if w not in first_mm_of_win:
                first_mm_of_win[w] = k
            last_mm_of_win[w] = k
            k += 1
    mm_total = k

    bf16 = mybir.dt.bfloat16
    f32 = mybir.dt.float32

    nc = bass.Bass(num_swdge_queues=4, dynamic_dma_scratch_size=scratch)
    wz = nc.declare_dram_parameter("wz", [n_rows, D], bf16, isOutput=False)
    idx = nc.declare_dram_parameter("idx", [P, ic], mybir.dt.int16, isOutput=False)
    seg = nc.declare_dram_parameter("seg", [P, sc], bf16, isOutput=False)
    iota = nc.declare_dram_parameter("iota", [P, P], bf16, isOutput=False)
    out = nc.declare_dram_parameter("out", [n_win * P, D], f32, isOutput=True)

    with contextlib.ExitStack() as ctx:
        idx_sb = ctx.enter_context(nc.sbuf_tensor([P, ic], mybir.dt.int16))
        seg_sb = ctx.enter_context(nc.sbuf_tensor([P, sc], bf16))
        iota_sb = ctx.enter_context(nc.sbuf_tensor([P, P], bf16))
        gbuf = ctx.enter_context(
            nc.sbuf_tensor([P, nbuf * region_slots * D], bf16)
        )
        ohbuf = ctx.enter_context(
            nc.sbuf_tensor([P, ohb * call_mm_max * P], bf16)
        )
        obuf = ctx.enter_context(nc.sbuf_tensor([P, o_bufs * D], f32))
        # One PSUM BANK per in-flight window: a matmul's start=True resets
        # the whole bank, so windows must not share banks while accumulating.
        assert G <= 4
        psums = [
            ctx.enter_context(nc.psum_tensor(f"ps{i}", [P, P], f32))
            for i in range(2 * G)
        ]
        in_sem = ctx.enter_context(nc.semaphore("in_sem"))
        isem = ctx.enter_context(nc.semaphore("isem"))
        # One gather-completion sem per (region, call-in-group) so at most
        # ONE DMA is ever in flight per sem (per-engine increments from two
        # concurrent DMAs interleave, so cumulative ge-16k waits on a shared
        # sem are racy).  Region gating (mmsem) bounds in-flight per sem to 1.
        n_gsem = nbuf * n_per_group
        gsems = [ctx.enter_context(nc.semaphore(f"gsem{i}")) for i in range(n_gsem)]
        ohsem = ctx.enter_context(nc.semaphore("ohsem"))
        mmsem = ctx.enter_context(nc.semaphore("mmsem"))
        csem = ctx.enter_context(nc.semaphore("csem"))
        ssems = [ctx.enter_context(nc.semaphore(f"ssem{i}")) for i in range(o_bufs)]
        block = ctx.enter_context(nc.Block())

        def gslot(j, s):
            """SBUF tile [P, D] of slot s of call j."""
            g = calls[j]["group"]
            base = (g % nbuf) * region_slots * D + (call_goff[j] + s) * D
            return gbuf[:, base : base + D]

        def gdest(j):
            g = calls[j]["group"]
            base = (g % nbuf) * region_slots * D + call_goff[j] * D
            n_slots = calls[j]["n_slots"]
            return gbuf[:, base : base + n_slots * D].rearrange(
                "p (s e) -> p s e", e=D
            )

        def ohcol(m_global, j):
            r = j % ohb
            off = (m_global - mm_cum[j]) * P
            return ohbuf[:, r * call_mm_max * P + off : r * call_mm_max * P + off + P]

        @block.sync
        def _(sync):
            sync.dma_start(idx_sb[:], idx[:]).then_inc(isem, 16)
            sync.dma_start(seg_sb[:], seg[:]).then_inc(in_sem, 16)
            sync.dma_start(iota_sb[:], iota[:]).then_inc(in_sem, 16)
            for w in range(n_win):
                sync.wait_ge(csem, w + 1)
                sync.dma_start(
                    out[w * P : (w + 1) * P, :],
                    obuf[:, (w % o_bufs) * D : (w % o_bufs + 1) * D],
                ).then_inc(ssems[w % o_bufs], 16)
            for lane in range(o_bufs):
                n_l = len(range(lane, n_win, o_bufs))
                if n_l:
                    sync.wait_ge(ssems[lane], 16 * n_l)

        @block.gpsimd
        def _(g):
            g.load_library(library_config.mlp)
            g.wait_ge(isem, 16)
            reg_ctx = g.register("ni_reg")
            ni = reg_ctx.__enter__()
            icol = 0
            for j, c in enumerate(calls):
                grp = c["group"]
                if j % n_per_group == 0 and grp >= nbuf:
                    g.wait_ge(mmsem, int(mm_cum[(grp - nbuf + 1) * n_per_group]))
                # nvalid differs per table but the SPMD program is shared,
                # so pads point at row 0 (valid) and every core gathers p16.
                g.reg_mov(ni, c["p16"])
                g.dma_gather(
                    out_ap=gdest(j),
                    in_ap=wz[c["chunk"] * chunk : min((c["chunk"] + 1) * chunk, n_rows), :],
                    idxs_ap=idx_sb[:, icol : icol + c["p16"] // 16],
                    num_idxs=c["p16"],
                    num_idxs_reg=ni,
                    elem_size=D,
                    single_packet=False,
                    queue_num=q_of_call[j],
                ).then_inc(gsems[j % n_gsem], 16)
                icol += c["p16"] // 16

        @block.vector
        def _(v):
            v.wait_ge(in_sem, 32)
            for j, c in enumerate(calls):
                n_mm = call_mm[j]
                if n_mm == 0:
                    continue
                if j >= ohb:
                    v.wait_ge(mmsem, int(mm_cum[j - ohb + 1]))
                r = j % ohb
                o = ohbuf[
                    :, r * call_mm_max * P : r * call_mm_max * P + n_mm * P
                ].rearrange("p (m e) -> p m e", e=P)
                s_in = (
                    seg_sb[:, mm_cum[j] : mm_cum[j + 1]]
                    .rearrange("p (m o) -> p m o", o=1)
                    .broadcast_to([P, n_mm, P])
                )
                i_in = (
                    iota_sb[:]
                    .rearrange("p (o e) -> p o e", o=1)
                    .broadcast_to([P, n_mm, P])
                )
                v.tensor_tensor(
                    out=o, in0=s_in, in1=i_in, op=mybir.AluOpType.is_equal
                ).then_inc(ohsem, 1)

        @block.tensor
        def _(pe):
            m_global = 0
            for j, c in enumerate(calls):
                grp = c["group"]
                if j % n_per_group == 0 and grp >= 2:
                    # psum region (grp % 2) free when group grp-2 fully copied
                    pe.wait_ge(csem, (grp - 1) * G)
                pe.wait_ge(gsems[j % n_gsem], 16 * (j // n_gsem + 1))
                if call_mm[j]:
                    pe.wait_ge(ohsem, sum(1 for jj in range(j + 1) if call_mm[jj]))
                for mi, (s, wloc) in enumerate(c["mms"]):
                    st, sp = c["flags"][mi]
                    pe.matmul(
                        psums[(grp % 2) * G + wloc][:],
                        lhsT=ohcol(m_global, j),
                        rhs=gslot(j, s),
                        start=st,
                        stop=sp,
                        skip_group_check=True,
                    ).then_inc(mmsem, 1)
                    m_global += 1

        @block.scalar
        def _(a):
            for w in range(n_win):
                a.wait_ge(mmsem, int(last_mm_of_win[w]) + 1)
                if w >= o_bufs:
                    wp = w - o_bufs
                    a.wait_ge(ssems[wp % o_bufs], 16 * (wp // o_bufs + 1))
                grp = w // G
                wloc = w % G
                a.copy(
                    obuf[:, (w % o_bufs) * D : (w % o_bufs + 1) * D],
                    psums[(grp % 2) * G + wloc][:],
                ).then_inc(csem, 1)

    return nc


def _run(weights, indices, offsets, trace=False, G=4, scratch=32768):
    import ml_dtypes
    from concourse import mybir
    from concourse.bass_utils import run_bass_kernel_spmd

    weights = np.asarray(weights)
    t, n, d = weights.shape
    assert d == D

    b = np.asarray(offsets).shape[1]
    n_win = b // P
    while G > 1 and n_win % G:
        G -= 1
    plan = _plan3(indices, offsets, n, G=G)

    # pads must gather a real row (see note in gpsimd block): rewrite -1
    # pads in idxbuf to 0.
    idxbuf = plan["idxbuf"].copy()
    idxbuf[idxbuf < 0] = 0

    wz16 = weights.astype(ml_dtypes.bfloat16)

    nc = _build_program3(n, plan, scratch=scratch)
    mybir.codegen_inst_isa_subclasses(nc)
    in_maps = [
        {
            "wz": np.ascontiguousarray(wz16[i]),
            "idx": np.ascontiguousarray(idxbuf[i]),
            "seg": np.ascontiguousarray(plan["segbuf"][i]),
            "iota": plan["iota"],
        }
        for i in range(t)
    ]
    res = run_bass_kernel_spmd(nc, in_maps, list(range(t)), trace=trace)
    out = np.stack([res.results[i]["out"] for i in range(t)], axis=0)
    return out, res


def kernel(weights, indices, offsets):
    out, _ = _run(weights, indices, offsets)
    return out


# revision 20
# speedup vs baseline: 1.8339x; 1.0351x over previous
"""MergedEmbeddingBag forward (sum pooling) on 8 Trainium2 NeuronCores.

Strategy (table-parallel, per sharding hint): core t owns table t.

v3 pipeline per core:
  - weights are cast to bf16 on the host and uploaded as [N, 128] bf16, so
    each gathered row is 256 B (half the HBM traffic of fp32) and feeds
    1-pass bf16 matmuls.
  - indices are planned on the host into supergroups of G windows (a window
    = 128 bags); within a supergroup, items are split by table chunk
    (N <= 4 chunks of <= 32767 rows so local indices fit signed int16) and
    gathered with one dma_gather per (supergroup, chunk) - few large SWDGE
    calls instead of many small ones.  Trailing pads use index -1 (skipped
    by HW, no traffic).
  - pooling: for each 128-row slot of gathered data, a one-hot bf16 matrix
    (generated on DVE with a single batched is_equal per call) scatters the
    rows into their bags via a PE matmul accumulated in PSUM.  Slots that
    span window boundaries are matmul'd once per window with masked one-hot
    columns, so there is no per-window padding of the gather stream.
  - scalar (ACT) copies finished PSUM windows to SBUF; sync (HWDGE) stores
    them to HBM.
"""

import sys

sys.path.insert(0, "/opt/trn_rl_repo")

import numpy as np

P = 128  # partitions / bags per window
D = 128  # embedding dim


def _plan3(indices, offsets, n_rows, G=4, max_chunk=32767):
    """Host planning: supergrouped, chunk-split, window-sorted gather order.

    Returns dict with device buffers (idxbuf int16, segbuf bf16 as uint16
    view, iota) and the static schedule consumed by _build_program3.
    """
    import ml_dtypes

    idx64 = np.ascontiguousarray(indices).astype(np.int64)
    off = np.ascontiguousarray(offsets).astype(np.int64)
    t, total = idx64.shape
    b = off.shape[1]
    assert b % P == 0
    n_win = b // P
    assert n_win % G == 0
    n_groups = n_win // G
    n_chunks = -(-n_rows // max_chunk)
    chunk = -(-n_rows // n_chunks)
    assert chunk <= max_chunk

    ends = np.concatenate([off[:, 1:], np.full((t, 1), total, np.int64)], axis=1)
    lens = np.clip(ends - off, 0, None)  # [T, B]

    l_uni = total // b
    uniform = (
        total == b * l_uni
        and (lens == l_uni).all()
        and (off == np.arange(b, dtype=np.int64) * l_uni).all()
    )

    # Per-table flat item lists with window-local bag and window ids, in
    # (window, bag, position) order.
    plans = []
    for i in range(t):
        if uniform:
            vals = idx64[i]  # already (bag, l) order; bags in window order
            bag = (np.arange(total) // l_uni) % P
            win = np.arange(total) // (P * l_uni)
        else:
            ls = lens[i]
            bag_of_item = np.repeat(np.arange(b), ls)
            pos = np.concatenate(
                [np.arange(off[i, j], off[i, j] + ls[j]) for j in range(b)]
            ) if ls.sum() else np.zeros(0, np.int64)
            vals = idx64[i, pos]
            bag = bag_of_item % P
            win = bag_of_item // P
        plans.append((vals, bag, win))

    # Shared static schedule across tables: per (group, chunk) call sizes
    # must be identical for the single SPMD program, so take the max and
    # pad with -1 (skipped).  mm schedule must also be shared: a matmul
    # exists for (call, slot, window) if ANY table has items there; its
    # seg column is per-table data.
    calls = []  # static: per call dict
    idx_cols = []  # per call: [T, p16] int16 (pad -1)
    seg_cols = []  # per call: [T, n_mm, 128] float (pad -1)
    for g in range(n_groups):
        w0, w1 = g * G, (g + 1) * G
        for c in range(n_chunks):
            per_t = []
            for i in range(t):
                vals, bag, win = plans[i]
                m = (win >= w0) & (win < w1) & (vals >= c * chunk) & (
                    vals < (c + 1) * chunk
                )
                per_t.append(
                    (
                        (vals[m] - c * chunk).astype(np.int16),
                        bag[m].astype(np.int16),
                        (win[m] - w0).astype(np.int16),
                    )
                )
            nmax = max(len(v) for v, _, _ in per_t)
            nmax = max(nmax, 1)
            # pad to FULL slots (pads rewritten to row 0 later): every gbuf
            # byte that a matmul can read gets written by its own call, so no
            # stale-NaN risk and no startup memset is needed.
            n_slots = -(-nmax // P)
            p16 = n_slots * P
            iv = np.full((t, p16), -1, np.int16)
            bv = np.full((t, n_slots * P), -1, np.int16)
            wv = np.full((t, n_slots * P), -2, np.int16)
            nvalid = np.zeros(t, np.int64)
            for i in range(t):
                v, bg, wn = per_t[i]
                k = len(v)
                iv[i, :k] = v
                bv[i, :k] = bg
                wv[i, :k] = wn
                nvalid[i] = k
            # mm list: per slot, windows present in ANY table (ascending)
            mms = []
            segs = []
            for s in range(n_slots):
                sl = slice(s * P, (s + 1) * P)
                wins_here = np.unique(wv[:, sl])
                wins_here = wins_here[wins_here >= 0]
                for wloc in wins_here:
                    seg = np.where(wv[:, sl] == wloc, bv[:, sl], -1).astype(
                        np.float32
                    )  # [T, 128]
                    mms.append((s, int(wloc)))
                    segs.append(seg)
            calls.append(
                dict(
                    group=g,
                    chunk=c,
                    p16=p16,
                    n_slots=n_slots,
                    nvalid=nvalid,
                    mms=mms,
                )
            )
            idx_cols.append(iv)
            seg_cols.append(
                np.stack(segs, axis=1) if segs else np.zeros((t, 0, P), np.float32)
            )

    # start/stop flags per matmul: per group, first/last mm of each window.
    n_per_group = len(calls) // n_groups
    mm_global = 0
    for g in range(n_groups):
        seen = {}
        order = []  # (call_idx, mm_idx, wloc)
        for cc in range(g * n_per_group, (g + 1) * n_per_group):
            for mi, (s, wloc) in enumerate(calls[cc]["mms"]):
                order.append((cc, mi, wloc))
        firsts, lasts = {}, {}
        for k, (cc, mi, wloc) in enumerate(order):
            if wloc not in firsts:
                firsts[wloc] = (cc, mi)
            lasts[wloc] = (cc, mi)
        for cc in range(g * n_per_group, (g + 1) * n_per_group):
            flags = []
            for mi, (s, wloc) in enumerate(calls[cc]["mms"]):
                flags.append(
                    (
                        firsts[wloc] == (cc, mi),
                        lasts[wloc] == (cc, mi),
                    )
                )
            calls[cc]["flags"] = flags
        # every window in the group must have at least one mm (else its
        # psum region is never written); guaranteed here because every
        # window has >= 1 item in >= 1 chunk.  Guard anyway:
        assert len(firsts) == G or b == 0, (g, sorted(firsts))
        mm_global += len(order)

    # Device buffers.
    # idxbuf: concat per-call [p16] wrapped to [16, p16/16], tiled to 128.
    iparts = []
    for iv in idx_cols:
        p16 = iv.shape[1]
        iparts.append(iv.reshape(t, p16 // 16, 16).transpose(0, 2, 1))
    idxbuf16 = np.concatenate(iparts, axis=2)  # [T, 16, IC]
    idxbuf = np.ascontiguousarray(np.tile(idxbuf16, (1, 8, 1)))  # [T, 128, IC]

    # segbuf: [T, 128, M_total] bf16 (partition p = slot row)
    sparts = [sc.transpose(0, 2, 1) for sc in seg_cols]  # [T, 128, n_mm]
    segbuf = np.concatenate(sparts, axis=2).astype(ml_dtypes.bfloat16)

    iota = np.tile(
        np.arange(P, dtype=np.float32)[None, :], (P, 1)
    ).astype(ml_dtypes.bfloat16)

    return dict(
        calls=calls,
        idxbuf=idxbuf,
        segbuf=np.ascontiguousarray(segbuf),
        iota=iota,
        chunk=chunk,
        n_chunks=n_chunks,
        G=G,
        n_groups=n_groups,
        n_win=n_win,
    )


def _build_program3(n_rows, plan, nbuf=2, ohb=10, o_bufs=4, scratch=16384):
    """Raw-Bass SPMD program for the v3 pipeline."""
    import contextlib

    import concourse.bass as bass
    import concourse.mybir as mybir
    from concourse import library_config

    calls = plan["calls"]
    chunk = plan["chunk"]
    G = plan["G"]
    n_groups = plan["n_groups"]
    n_win = plan["n_win"]
    n_calls = len(calls)
    n_per_group = n_calls // n_groups
    ic = plan["idxbuf"].shape[2]
    sc = max(1, plan["segbuf"].shape[2])

    # static derived counts
    call_mm = [len(c["mms"]) for c in calls]
    mm_cum = np.cumsum([0] + call_mm)  # mm count before call j
    group_slots = [
        sum(calls[cc]["n_slots"] for cc in range(g * n_per_group, (g + 1) * n_per_group))
        for g in range(n_groups)
    ]
    region_slots = max(group_slots)
    call_mm_max = max(call_mm)
    # group gbuf slot offsets per call
    call_goff = []
    for g in range(n_groups):
        o = 0
        for cc in range(g * n_per_group, (g + 1) * n_per_group):
            call_goff.append(o)
            o += calls[cc]["n_slots"]
    # per-queue cumulative call counts
    q_of_call = [j % 4 for j in range(n_calls)]
    q_count_after = []
    qc = [0, 0, 0, 0]
    for j in range(n_calls):
        qc[q_of_call[j]] += 1
        q_count_after.append(qc[q_of_call[j]])
    # mm index of each window's last matmul (global, in issue order)
    last_mm_of_win = {}
    first_mm_of_win = {}
    k = 0
    for j, c in enumerate(calls):
        g = c["group"]
        for (s, wloc) in c["mms"]:
            w = g * G + wloc
            if w not in first_mm_of_win:
                first_mm_of_win[w] = k
            last_mm_of_win[w] = k
            k += 1
    mm_total = k

    bf16 = mybir.dt.bfloat16
    f32 = mybir.dt.float32

    nc = bass.Bass(num_swdge_queues=4, dynamic_dma_scratch_size=scratch)
    wz = nc.declare_dram_parameter("wz", [n_rows, D], bf16, isOutput=False)
    idx = nc.declare_dram_parameter("idx", [P, ic], mybir.dt.int16, isOutput=False)
    seg = nc.declare_dram_parameter("seg", [P, sc], bf16, isOutput=False)
    iota = nc.declare_dram_parameter("iota", [P, P], bf16, isOutput=False)
    out = nc.declare_dram_parameter("out", [n_win * P, D], f32, isOutput=True)

    with contextlib.ExitStack() as ctx:
        idx_sb = ctx.enter_context(nc.sbuf_tensor([P, ic], mybir.dt.int16))
        seg_sb = ctx.enter_context(nc.sbuf_tensor([P, sc], bf16))
        iota_sb = ctx.enter_context(nc.sbuf_tensor([P, P], bf16))
        gbuf = ctx.enter_context(
            nc.sbuf_tensor([P, nbuf * region_slots * D], bf16)
        )
        ohbuf = ctx.enter_context(
            nc.sbuf_tensor([P, ohb * call_mm_max * P], bf16)
        )
        obuf = ctx.enter_context(nc.sbuf_tensor([P, o_bufs * D], f32))
        # One PSUM BANK per in-flight window: a matmul's start=True resets
        # the whole bank, so windows must not share banks while accumulating.
        assert G <= 4
        psums = [
            ctx.enter_context(nc.psum_tensor(f"ps{i}", [P, P], f32))
            for i in range(2 * G)
        ]
        in_sem = ctx.enter_context(nc.semaphore("in_sem"))
        isem = ctx.enter_context(nc.semaphore("isem"))
        # One gather-completion sem per (region, call-in-group) so at most
        # ONE DMA is ever in flight per sem (per-engine increments from two
        # concurrent DMAs interleave, so cumulative ge-16k waits on a shared
        # sem are racy).  Region gating (mmsem) bounds in-flight per sem to 1.
        n_gsem = nbuf * n_per_group
        gsems = [ctx.enter_context(nc.semaphore(f"gsem{i}")) for i in range(n_gsem)]
        ohsem = ctx.enter_context(nc.semaphore("ohsem"))
        mmsem = ctx.enter_context(nc.semaphore("mmsem"))
        csem = ctx.enter_context(nc.semaphore("csem"))
        ssems = [ctx.enter_context(nc.semaphore(f"ssem{i}")) for i in range(o_bufs)]
        block = ctx.enter_context(nc.Block())

        def gslot(j, s):
            """SBUF tile [P, D] of slot s of call j."""
            g = calls[j]["group"]
            base = (g % nbuf) * region_slots * D + (call_goff[j] + s) * D
            return gbuf[:, base : base + D]

        def gdest(j):
            g = calls[j]["group"]
            base = (g % nbuf) * region_slots * D + call_goff[j] * D
            n_slots = calls[j]["n_slots"]
            return gbuf[:, base : base + n_slots * D].rearrange(
                "p (s e) -> p s e", e=D
            )

        def ohcol(m_global, j):
            r = j % ohb
            off = (m_global - mm_cum[j]) * P
            return ohbuf[:, r * call_mm_max * P + off : r * call_mm_max * P + off + P]

        # idx columns needed by the first nbuf groups' calls (prefix load
        # lets gathers start before the full idx buffer has landed)
        icol_pref = sum(calls[j]["p16"] // 16 for j in range(min(nbuf * n_per_group, n_calls)))

        @block.sync
        def _(sync):
            sync.dma_start(idx_sb[:, :icol_pref], idx[:, :icol_pref]).then_inc(isem, 16)
            if icol_pref < ic:
                sync.dma_start(idx_sb[:, icol_pref:], idx[:, icol_pref:]).then_inc(isem, 16)
            sync.dma_start(seg_sb[:], seg[:]).then_inc(in_sem, 16)
            sync.dma_start(iota_sb[:], iota[:]).then_inc(in_sem, 16)
            for w in range(n_win):
                sync.wait_ge(csem, w + 1)
                sync.dma_start(
                    out[w * P : (w + 1) * P, :],
                    obuf[:, (w % o_bufs) * D : (w % o_bufs + 1) * D],
                ).then_inc(ssems[w % o_bufs], 16)
            for lane in range(o_bufs):
                n_l = len(range(lane, n_win, o_bufs))
                if n_l:
                    sync.wait_ge(ssems[lane], 16 * n_l)

        @block.gpsimd
        def _(g):
            g.load_library(library_config.mlp)
            g.wait_ge(isem, 16)
            full_idx_waited = icol_pref >= ic
            reg_ctx = g.register("ni_reg")
            ni = reg_ctx.__enter__()
            icol = 0
            for j, c in enumerate(calls):
                grp = c["group"]
                if not full_idx_waited and j >= nbuf * n_per_group:
                    g.wait_ge(isem, 32)
                    full_idx_waited = True
                if j % n_per_group == 0 and grp >= nbuf:
                    g.wait_ge(mmsem, int(mm_cum[(grp - nbuf + 1) * n_per_group]))
                # nvalid differs per table but the SPMD program is shared,
                # so pads point at row 0 (valid) and every core gathers p16.
                g.reg_mov(ni, c["p16"])
                g.dma_gather(
                    out_ap=gdest(j),
                    in_ap=wz[c["chunk"] * chunk : min((c["chunk"] + 1) * chunk, n_rows), :],
                    idxs_ap=idx_sb[:, icol : icol + c["p16"] // 16],
                    num_idxs=c["p16"],
                    num_idxs_reg=ni,
                    elem_size=D,
                    single_packet=False,
                    queue_num=q_of_call[j],
                ).then_inc(gsems[j % n_gsem], 16)
                icol += c["p16"] // 16

        @block.vector
        def _(v):
            v.wait_ge(in_sem, 32)
            for j, c in enumerate(calls):
                n_mm = call_mm[j]
                if n_mm == 0:
                    continue
                if j >= ohb:
                    v.wait_ge(mmsem, int(mm_cum[j - ohb + 1]))
                r = j % ohb
                o = ohbuf[
                    :, r * call_mm_max * P : r * call_mm_max * P + n_mm * P
                ].rearrange("p (m e) -> p m e", e=P)
                s_in = (
                    seg_sb[:, mm_cum[j] : mm_cum[j + 1]]
                    .rearrange("p (m o) -> p m o", o=1)
                    .broadcast_to([P, n_mm, P])
                )
                i_in = (
                    iota_sb[:]
                    .rearrange("p (o e) -> p o e", o=1)
                    .broadcast_to([P, n_mm, P])
                )
                v.tensor_tensor(
                    out=o, in0=s_in, in1=i_in, op=mybir.AluOpType.is_equal
                ).then_inc(ohsem, 1)

        @block.tensor
        def _(pe):
            m_global = 0
            for j, c in enumerate(calls):
                grp = c["group"]
                if j % n_per_group == 0 and grp >= 2:
                    # psum region (grp % 2) free when group grp-2 fully copied
                    pe.wait_ge(csem, (grp - 1) * G)
                pe.wait_ge(gsems[j % n_gsem], 16 * (j // n_gsem + 1))
                if call_mm[j]:
                    pe.wait_ge(ohsem, sum(1 for jj in range(j + 1) if call_mm[jj]))
                for mi, (s, wloc) in enumerate(c["mms"]):
                    st, sp = c["flags"][mi]
                    pe.matmul(
                        psums[(grp % 2) * G + wloc][:],
                        lhsT=ohcol(m_global, j),
                        rhs=gslot(j, s),
                        start=st,
                        stop=sp,
                        skip_group_check=True,
                    ).then_inc(mmsem, 1)
                    m_global += 1

        @block.scalar
        def _(a):
            for w in range(n_win):
                a.wait_ge(mmsem, int(last_mm_of_win[w]) + 1)
                if w >= o_bufs:
                    wp = w - o_bufs
                    a.wait_ge(ssems[wp % o_bufs], 16 * (wp // o_bufs + 1))
                grp = w // G
                wloc = w % G
                a.copy(
                    obuf[:, (w % o_bufs) * D : (w % o_bufs + 1) * D],
                    psums[(grp % 2) * G + wloc][:],
                ).then_inc(csem, 1)

    return nc


def _run(weights, indices, offsets, trace=False, G=4, scratch=16384):
    import ml_dtypes
    from concourse import mybir
    from concourse.bass_utils import run_bass_kernel_spmd

    weights = np.asarray(weights)
    t, n, d = weights.shape
    assert d == D

    b = np.asarray(offsets).shape[1]
    n_win = b // P
    while G > 1 and n_win % G:
        G -= 1
    plan = _plan3(indices, offsets, n, G=G)

    # pads must gather a real row (see note in gpsimd block): rewrite -1
    # pads in idxbuf to 0.
    idxbuf = plan["idxbuf"].copy()
    idxbuf[idxbuf < 0] = 0

    wz16 = weights.astype(ml_dtypes.bfloat16)

    nc = _build_program3(n, plan, scratch=scratch)
    mybir.codegen_inst_isa_subclasses(nc)
    in_maps = [
        {
            "wz": np.ascontiguousarray(wz16[i]),
            "idx": np.ascontiguousarray(idxbuf[i]),
            "seg": np.ascontiguousarray(plan["segbuf"][i]),
            "iota": plan["iota"],
        }
        for i in range(t)
    ]
    res = run_bass_kernel_spmd(nc, in_maps, list(range(t)), trace=trace)
    out = np.stack([res.results[i]["out"] for i in range(t)], axis=0)
    return out, res


def kernel(weights, indices, offsets):
    out, _ = _run(weights, indices, offsets)
    return out
